# revision 8
# baseline (speedup 1.0000x reference)
"""AutoCorrelation kernel for Trainium2 (8 NeuronCores, SPMD data-parallel over batch).

Math (derived from the reference nn.Module):
  - R = irfft(rfft(Q) * conj(rfft(K))) is a circular cross-correlation; the
    reference reduces it with mean over (heads, ALL lags).  Sum over all lags
    of a circular cross-correlation factorizes:  sum_tau R[tau] =
    (sum_t Q[t]) * (sum_s K[s]).  So the FFT is algebraically unnecessary --
    only column sums of Q and K are needed, and those are linear in the
    column sums of q and k (sum_t(q @ Wq + bq) = (sum_t q) @ Wq + L*bq).
  - The top-k "delays" are channel indices in [0, 64).  The delay aggregation
    sum_i w_i * roll(V, -d_i) commutes with the output projection AND with the
    value projection, so:  out[t] = sum_d coef_d * U[(t+d) % L]  where
    U = v @ (Wv @ Wo), plus bias (bv @ Wo + bo).  Because sum_d coef_d = 1
    (softmax weights), the bias folds into U:  out[t] = sum_d coef_d *
    (U + bias)[(t+d) % L].  The tap sum is a 64-band Toeplitz matmul.

Device work:
  phase 1: column sums of q[b], k[b] per core via ones-vector matmuls in
           fp8-e4m3 DoubleRow perf mode (errors average out over the
           4096-element sums).  8 x 512KB chunks alternating the two DMA
           rings, with the reduction matmuls issued per-chunk so the PE
           trails the last DMA by only ~2 matmuls.
  phase 2: U = v @ W2 + bias per 128-row tile (bf16 matmuls, fp32 PSUM,
           DVE bias-add + downcast), then out_j = band1^T U_j + band2^T
           U_{j+1} (circular), stored as bf16.  The PE stream interleaves
           U-tile production ~6 tiles ahead of the banded conv so the
           tensor engine never waits once the first vT chunk lands.
Host work: [8,512]@[512,512] glue matmuls, top-41 of 64, softmax, band build.

Measured facts this schedule is built on (see traces/):
  - a 512-free matmul sustains ~220ns (=512 cycles @2.4GHz) with ldweights
    fully hidden, for bf16 and fp8 alike -> phase-2 PE floor = 192 matmuls
    ~= 42.3us; everything else must hide under it.
  - fp8 anywhere on the value path costs ~2-6e-2 rel err (output absmax is
    only 0.24) -> value path stays bf16; fp8 only feeds the top-k glue.
  - per-NEFF fixed overhead (preamble+epilogue) ~= 17us; collectives add
    ~65us of cross-core dispatch skew under this harness -> keep exactly
    two launches with host glue between.
"""

import sys

sys.path.insert(0, "/opt/trn_rl_repo")

import numpy as np

import concourse.bass as bass
import concourse.bacc as bacc
import concourse.mybir as mybir
import concourse.tile as tile
from concourse.bass_utils import run_bass_kernel_spmd

B, L, D, H = 8, 4096, 512, 8
DK = D // H          # 64
K_TOP = 41           # min(int(5*log(4096)), 64)
NCORES = 8
F32 = mybir.dt.float32
BF16 = mybir.dt.bfloat16
FP8 = mybir.dt.float8e4
NP_BF16 = mybir.dt.np(BF16)
NP_FP8 = mybir.dt.np(FP8)

# set by test.py to collect HW profiles
PROFILE = False
TRACE_DIR = None
LAST_HW_TIME_NS = {"phase1": None, "phase2": None}

_NC_CACHE = {}


def _make_nc():
    return bacc.Bacc(
        "TRN2", target_bir_lowering=False, debug=False, num_devices=NCORES
    )


def _build_phase1():
    """Per-core: sums[0, :512] = sum_t q[t, :], sums[0, 512:] = sum_t k[t, :].

    q/k arrive as fp8 e4m3 (2 MB each); sums accumulate in fp32 PSUM via
    DoubleRow ones-vector matmuls (2 contraction rows/cycle).  Each tensor is
    four 512 KB DMAs interleaved across the two rings; the 4 reduction
    matmuls for a chunk are issued right behind that chunk's DMA.
    """
    nc = _make_nc()
    I8 = mybir.dt.int8
    q = nc.dram_tensor("q", [L, D], I8, kind="ExternalInput")
    k = nc.dram_tensor("k", [L, D], I8, kind="ExternalInput")
    sums = nc.dram_tensor("sums", [1, 2 * D], F32, kind="ExternalOutput")

    NCH = 4                   # DMA chunks per tensor (512 KB each, 4 KB/partition)
    NSUB = 8                  # row-groups of 128 per chunk
    DR = mybir.MatmulPerfMode.DoubleRow

    with tile.TileContext(nc) as tc:
        with (
            tc.tile_pool(name="singles", bufs=1) as singles,
            tc.tile_pool(name="ps", bufs=2, space=bass.MemorySpace.PSUM) as ps_pool,
        ):
            # [128, 2, 16]: DoubleRow ldweights wants 16B outer free stride
            ones = singles.tile([128, 2, 16], FP8)
            nc.any.memset(ones[:], 1.0)

            q_re = q.ap().rearrange("(h p n) d -> h p n d", p=128, n=NSUB)
            k_re = k.ap().rearrange("(h p n) d -> h p n d", p=128, n=NSUB)

            qt = [singles.tile([128, NSUB, D], I8, name=f"qt{h}") for h in range(NCH)]
            kt = [singles.tile([128, NSUB, D], I8, name=f"kt{h}") for h in range(NCH)]
            # sync's queue starts ~1.6us earlier and runs ~1.4x faster than
            # scalar's, so give it 5 of the 8 chunks; interleave q/k so the
            # per-chunk reduction matmuls can start after the first 512 KB
            chunks = [("k", 0), ("q", 0), ("k", 1), ("q", 1),
                      ("k", 2), ("q", 2), ("k", 3), ("q", 3)]
            rings = [nc.sync, nc.sync, nc.scalar, nc.sync,
                     nc.scalar, nc.sync, nc.scalar, nc.sync]
            for (which, h), ring in zip(chunks, rings):
                t, re = (kt, k_re) if which == "k" else (qt, q_re)
                ring.dma_start(t[h][:], re[h])

            psq = ps_pool.tile([1, D], F32)
            psk = ps_pool.tile([1, D], F32)

            def colsum(ps, t, first, last):
                for c in range(0, NSUB, 2):
                    nc.tensor.matmul(
                        ps[:1, :],
                        ones[:, :, 0:1],
                        t[:, c : c + 2, :].bitcast(FP8),
                        start=(first and c == 0),
                        stop=(last and c == NSUB - 2),
                        perf_mode=DR,
                    )

            for h in range(NCH):
                colsum(psk, kt[h], h == 0, h == NCH - 1)
                colsum(psq, qt[h], h == 0, h == NCH - 1)

            osb = singles.tile([1, 2 * D], F32)
            nc.vector.tensor_copy(osb[:1, D : 2 * D], psk[:1, :])
            nc.vector.tensor_copy(osb[:1, 0:D], psq[:1, :])
            nc.scalar.dma_start(sums.ap(), osb[:])

    nc.compile()
    return nc


def _build_phase2():
    """Per-core: out[128j + r, n] = sum_s band1[s, r] * U_j[s, n]
                                  + sum_s band2[s, r] * U_{j+1 mod 32}[s, n]
    with U_j = v[128j : 128(j+1), :] @ W2 + bias (bias folded in since the
    softmax coefs sum to 1).  Output stored bf16.

    PE stream: 192 matmuls (128 U-proj + 64 conv) issued so that U-tile
    production runs LEAD tiles ahead of the conv that consumes them; with
    ~220ns per matmul the PE is the only critical resource.  DVE does the
    U drains (bias add, fp32 PSUM -> bf16 SBUF), ACT the out drains.
    """
    nc = _make_nc()
    vT = nc.dram_tensor("vT", [D, L], BF16, kind="ExternalInput")
    bandsd = nc.dram_tensor("bands", [2, 128, 128], BF16, kind="ExternalInput")
    # host-swizzled: w2[p, cg*D + n] = (Wv@Wo)[cg*128 + p, n] (contiguous rows)
    w2d = nc.dram_tensor("w2", [128, 4 * D], BF16, kind="ExternalInput")
    biasd = nc.dram_tensor("bias", [1, D], F32, kind="ExternalInput")
    out = nc.dram_tensor("out", [L, D], BF16, kind="ExternalOutput")

    NBLK = L // 128          # 32 U tiles / output blocks
    TCH = 512                # vT DMA chunk width (time cols); 4 U tiles each
    NTCH = L // TCH          # 8 chunks per channel group
    LEAD = 6                 # U tiles produced ahead of the conv
    OSUB = 2                 # output blocks per store DMA

    with tile.TileContext(nc) as tc:
        with (
            tc.tile_pool(name="singles", bufs=1) as singles,
            tc.tile_pool(name="osb", bufs=3) as opool,
            tc.tile_pool(name="ups", bufs=3, space=bass.MemorySpace.PSUM) as ups_pool,
            tc.tile_pool(name="ops", bufs=4, space=bass.MemorySpace.PSUM) as ops_pool,
        ):
            vt_re = vT.ap().rearrange("(c p) t -> c p t", p=128)
            vt = [singles.tile([128, L], BF16, name=f"vt{c}") for c in range(4)]
            w2_sb = singles.tile([128, 4, D], BF16)
            w2_re = w2d.ap().rearrange("p (c n) -> p c n", c=4)
            band_sb = singles.tile([128, 2, 128], BF16)
            bias_row = singles.tile([1, D], F32)
            bias_sb = singles.tile([128, D], F32)

            # DMA schedule: each dma_start costs ~0.7us to issue and the
            # per-engine DMA semaphore rotation is only 4 deep, so queue
            # throughput collapses when fed many small DMAs (~128KB chunks
            # measured ~110GB/s vs ~200-330GB/s for 512KB+).  So: 4 tiny
            # "starter" chunks let the first U tiles begin ~10.5us, and the
            # rest of vT moves in ~1MB pieces.  bands/bias are not needed
            # until the first conv (~6 tiles in).
            ST = 256                  # starter: t cols 0..255 (2 U tiles)
            MID = 2176
            for cg in range(4):
                ring = nc.sync if cg % 2 == 0 else nc.scalar
                ring.dma_start(w2_sb[:, cg, :], w2_re[:, cg, :])
                ring.dma_start(vt[cg][:, 0:ST], vt_re[cg][:, 0:ST])
            nc.sync.dma_start(band_sb[:], bandsd.ap().rearrange("b p t -> p b t"))
            nc.scalar.dma_start(bias_row[:], biasd.ap())
            nc.gpsimd.partition_broadcast(bias_sb[:], bias_row[:1, :])
            for cg in range(4):
                ring = nc.sync if cg % 2 == 0 else nc.scalar
                ring.dma_start(vt[cg][:, ST:MID], vt_re[cg][:, ST:MID])
            for cg in range(4):
                ring = nc.sync if cg % 2 == 0 else nc.scalar
                ring.dma_start(vt[cg][:, MID:L], vt_re[cg][:, MID:L])

            # U strip: 33 bf16 slots (slot 32 = wrap copy of U_0)
            ustrip = singles.tile([128, NBLK + 1, D], BF16)
            out_re = out.ap().rearrange("(g n p) d -> g p n d", p=128, n=OSUB)

            def u_tile(i):
                ups = ups_pool.tile([128, D], F32, tag="ups", name=f"ups{i}")
                for cg in range(4):
                    nc.tensor.matmul(
                        ups[:],
                        vt[cg][:, i * 128 : (i + 1) * 128],
                        w2_sb[:, cg, :],
                        start=(cg == 0),
                        stop=(cg == 3),
                    )
                # fp32 PSUM + fp32 bias -> bf16 SBUF on the DVE
                nc.vector.tensor_add(ustrip[:, i, :], ups[:], bias_sb[:])
                if i == 0:
                    nc.vector.tensor_add(ustrip[:, NBLK, :], ups[:], bias_sb[:])

            for i in range(LEAD):
                u_tile(i)

            ot = None
            for j in range(NBLK):
                ops = ops_pool.tile([128, D], F32, tag="ops", name=f"ops{j}")
                nc.tensor.matmul(
                    ops[:], band_sb[:, 0, :], ustrip[:, j, :],
                    start=True, stop=False,
                )
                if j + LEAD < NBLK:
                    u_tile(j + LEAD)
                nc.tensor.matmul(
                    ops[:], band_sb[:, 1, :], ustrip[:, j + 1, :],
                    start=False, stop=True,
                )
                if j % OSUB == 0:
                    ot = opool.tile([128, OSUB, D], BF16, tag="out", name=f"ot{j // OSUB}")
                # out drains alternate ACT / DVE (GpSimd cannot read PSUM);
                # all out DMAs issue from sync (idle after inputs), keeping
                # ACT free of ~0.7us dma_start issue costs.
                if j % 2 == 0:
                    nc.scalar.copy(ot[:, j % OSUB, :], ops[:])
                else:
                    nc.vector.tensor_copy(ot[:, j % OSUB, :], ops[:])
                if j % OSUB == OSUB - 1:
                    nc.sync.dma_start(out_re[j // OSUB], ot[:])

    nc.compile()
    return nc


_RUN_COUNTER = [0]


def _run(nc, in_maps, phase):
    kwargs = {}
    if PROFILE:
        kwargs["trace"] = True
        if TRACE_DIR is not None:
            import os

            _RUN_COUNTER[0] += 1
            d = os.path.join(TRACE_DIR, f"{phase}_{_RUN_COUNTER[0]}")
            os.makedirs(d, exist_ok=True)
            kwargs["tmpdir"] = d
    res = run_bass_kernel_spmd(nc, in_maps, core_ids=list(range(NCORES)), **kwargs)
    LAST_HW_TIME_NS[phase] = res.exec_time_ns
    return res.results


def kernel(q, k, v, Wq, bq, Wk, bk, Wv, bv, Wo, bo):
    q = np.asarray(q, dtype=np.float32)
    k = np.asarray(k, dtype=np.float32)
    v = np.asarray(v, dtype=np.float32)
    Wq, bq, Wk, bk, Wv, bv, Wo, bo = (
        np.asarray(x, dtype=np.float64) for x in (Wq, bq, Wk, bk, Wv, bv, Wo, bo)
    )

    # ---- phase 1: per-batch column sums of q and k (device, fp8) ----
    if "p1" not in _NC_CACHE:
        _NC_CACHE["p1"] = _build_phase1()
    q_f8 = q.astype(NP_FP8).view(np.int8)
    k_f8 = k.astype(NP_FP8).view(np.int8)
    in_maps = [{"q": q_f8[b], "k": k_f8[b]} for b in range(B)]
    res1 = _run(_NC_CACHE["p1"], in_maps, "phase1")
    sq = np.stack([res1[b]["sums"][0, :D] for b in range(B)]).astype(np.float64)
    sk = np.stack([res1[b]["sums"][0, D:] for b in range(B)]).astype(np.float64)

    # ---- host glue: top-k channel selection + softmax weights ----
    SQ = sq @ Wq + L * bq                       # [B, D]
    SK = sk @ Wk + L * bk
    m = (SQ.reshape(B, H, DK) * SK.reshape(B, H, DK)).sum(axis=1) / (H * L)  # [B, DK]
    mbar = m.mean(axis=0)
    idx = np.argsort(-mbar, kind="stable")[:K_TOP]
    msel = m[:, idx]
    e = np.exp(msel - msel.max(axis=1, keepdims=True))
    w = e / e.sum(axis=1, keepdims=True)        # [B, K_TOP]
    coef = np.zeros((B, DK))
    coef[:, idx] = w

    # Toeplitz bands: out[t] = sum_d coef[d] * U[(t + d) % L]
    s = np.arange(128)[:, None]
    t = np.arange(128)[None, :]
    d1 = s - t
    d2 = s + 128 - t
    m1 = (d1 >= 0) & (d1 < DK)
    m2 = (d2 >= 0) & (d2 < DK)
    bands = np.zeros((B, 2, 128, 128), dtype=np.float64)
    for b in range(B):
        bands[b, 0] = np.where(m1, coef[b][np.clip(d1, 0, DK - 1)], 0.0)
        bands[b, 1] = np.where(m2, coef[b][np.clip(d2, 0, DK - 1)], 0.0)

    W2 = (Wv @ Wo).astype(np.float32)
    bias2 = (bv @ Wo + bo).astype(np.float32).reshape(1, D)
    # swizzle so W2 rows for channel chunk cg sit contiguously per partition
    w2_bf = np.ascontiguousarray(
        W2.reshape(4, 128, D).transpose(1, 0, 2).reshape(128, 4 * D)
    ).astype(NP_BF16)
    bands_bf = bands.astype(NP_BF16)
    vT_bf = np.ascontiguousarray(v.transpose(0, 2, 1)).astype(NP_BF16)  # [B, D, L]

    # ---- phase 2: folded projection + tap aggregation (device) ----
    if "p2" not in _NC_CACHE:
        _NC_CACHE["p2"] = _build_phase2()
    in_maps = [
        {
            "vT": vT_bf[b],
            "bands": np.ascontiguousarray(bands_bf[b]),
            "w2": w2_bf,
            "bias": bias2,
        }
        for b in range(B)
    ]
    res2 = _run(_NC_CACHE["p2"], in_maps, "phase2")
    return np.stack([res2[b]["out"].astype(np.float32) for b in range(B)])
